# revision 8
# baseline (speedup 1.0000x reference)
"""EVA-02 ViT attention block (LoRA + rope + rel-pos-bias) on 8 TRN2 NeuronCores.

Data-parallel over batch (8 images per core). Per core:
  - LoRA merged into qkv weights on the host; q-scale and v-bias folded away.
  - q/k projected in transposed layout (channels on partitions), v natural,
    fp32r matmuls at full PE rate (free dim >= 256 via image pairs).
  - rope in transposed layout; the pair swap is a DVE stream_shuffle (channels
    host-permuted so rope pairs sit 16 apart within 32-partition blocks).
  - scores transposed (S^T[j,i]); rel-pos bias added via an identity matmul
    into the same PSUM accumulation group; exp on ScalarE without max
    subtraction (scores are O(1)); probs bf16.
  - attn@v with v stationary emits O^T directly; softmax denominators via
    ones-vector matmuls; 1/x as exp(-ln x) on ScalarE; normalization applied
    in the O^T eviction multiply.
  - output projection back to natural [n, c] layout; bias via K=1 ones matmul.
"""
import numpy as np
import ml_dtypes

B, N, C, H, R = 64, 197, 768, 12, 24
D = C // H               # 64
NCORES = 8
BPC = B // NCORES        # images per core
F2 = 2 * N               # 394
F4 = 4 * N               # 788
N0, N1 = 128, N - 128    # token chunks: 128 + 69

_cache = {}

SHUF_MASK = list(range(16, 32)) + list(range(0, 16))


def _perm64():
    p = []
    for blk in range(2):
        base = blk * 32
        p += [base + 2 * t for t in range(16)]
        p += [base + 2 * t + 1 for t in range(16)]
    return np.array(p)


def _swap16_rows(a):
    out = np.empty_like(a)
    for s in range(a.shape[0] // 32):
        out[s * 32:s * 32 + 16] = a[s * 32 + 16:s * 32 + 32]
        out[s * 32 + 16:s * 32 + 32] = a[s * 32:s * 32 + 16]
    return out


def build_program(n_pairs=BPC // 2, use_shuffle=True, repeat=1):
    import concourse.bass as bass
    import concourse.tile as tile
    from concourse import bacc, mybir

    f32, f32r, bf16 = mybir.dt.float32, mybir.dt.float32r, mybir.dt.bfloat16
    AF = mybir.ActivationFunctionType
    OP = mybir.AluOpType

    nc = bacc.Bacc("TRN2", target_bir_lowering=False, debug=False)
    n_img = 2 * n_pairs

    xt_d = nc.dram_tensor("xt", [n_pairs, C, F2], f32, kind="ExternalInput")
    wt_d = nc.dram_tensor("wt", [C, 3 * C], f32, kind="ExternalInput")
    bq_d = nc.dram_tensor("bq", [128, 12], f32, kind="ExternalInput")
    bqs_d = nc.dram_tensor("bqs", [128, 12], f32, kind="ExternalInput")
    cs_d = nc.dram_tensor("cs", [2, 128, F4], bf16, kind="ExternalInput")
    erpb_d = nc.dram_tensor("erpbt", [6, 2, 128, F2], bf16, kind="ExternalInput")
    projt_d = nc.dram_tensor("projt", [C, C], f32, kind="ExternalInput")
    projb_d = nc.dram_tensor("projb", [1, C], f32, kind="ExternalInput")
    y_d = nc.dram_tensor("y", [n_img, N, C], f32, kind="ExternalOutput")

    from contextlib import ExitStack
    with tile.TileContext(nc) as tc:
        with ExitStack() as stk:
            pool = lambda name, bufs, **kw: stk.enter_context(
                tc.tile_pool(name=name, bufs=bufs, **kw))
            # NOTE: bufs is per-tag. PSUM budget: qkps 2 + vps 1 + psA 1 +
            # psB 1 + aops 1 + sums 1 + yps 1 = 8 banks exactly.
            constp = pool("const", 1)
            xtp = pool("xt", 2)
            qkps = pool("qkps", 2, space="PSUM")
            vps = pool("vps", 1, space="PSUM")
            qkbfp = pool("qkbf", 2)
            ropet = pool("ropet", 1)
            vsbp = pool("vsb", 8)
            scps = pool("scps", 1, space="PSUM")
            probsp = pool("probs", 4)
            aops = pool("aops", 1, space="PSUM")
            sumsp = pool("sums", 1, space="PSUM")
            rsbp = pool("rsb", 2)
            aosbp = pool("aosb", 6)
            rbc = pool("rbc", 2)
            yps = pool("yps", 1, space="PSUM")
            ysbp = pool("ysb", 2)
            otp = pool("otp", 6)

            # ---- constants (batched DMAs) ----
            wt_all = constp.tile([128, 6 * 3 * C], f32r, tag="wtall")
            nc.sync.dma_start(
                wt_all[:].rearrange("p (cc j) -> p cc j", cc=6),
                wt_d.rearrange("(cc p) j -> cc p j", cc=6)
                .transpose((1, 0, 2)).bitcast(f32r))
            wt_sb = [wt_all[:, cc * 3 * C:(cc + 1) * 3 * C] for cc in range(6)]
            pt_all = constp.tile([128, 6 * C], f32r, tag="ptall")
            nc.sync.dma_start(
                pt_all[:].rearrange("p (cc j) -> p cc j", cc=6),
                projt_d.rearrange("(cc p) j -> cc p j", cc=6)
                .transpose((1, 0, 2)).bitcast(f32r))
            projt_sb = [pt_all[:, cc * C:(cc + 1) * C] for cc in range(6)]
            erpb_all = constp.tile([128, 12 * F2], bf16, tag="erpball")
            nc.sync.dma_start(
                erpb_all[:].rearrange("p (g j) -> p g j", g=12),
                erpb_d.rearrange("h c p j -> (h c) p j").transpose((1, 0, 2)))
            erpb_sb = [(erpb_all[:, (2 * hp) * F2:(2 * hp + 1) * F2],
                        erpb_all[:, (2 * hp + 1) * F2:(2 * hp + 2) * F2])
                       for hp in range(6)]
            projb_bc = constp.tile([128, C], f32, tag="pbbc")
            nc.gpsimd.dma_start(
                projb_bc[:],
                projb_d[:].unsqueeze(1).broadcast_to((1, 128, C)))
            bq_sb = constp.tile([128, 12], f32, tag="bq")
            nc.sync.dma_start(bq_sb[:], bq_d[:])
            bqs_sb = constp.tile([128, 12], f32, tag="bqs")
            nc.sync.dma_start(bqs_sb[:], bqs_d[:])
            cos_sb = constp.tile([128, F4], bf16, tag="cos")
            nc.sync.dma_start(cos_sb[:], cs_d[0])
            spm_sb = constp.tile([128, F4], bf16, tag="spm")
            nc.sync.dma_start(spm_sb[:], cs_d[1])
            # E-band: column 11 is ones; slicing [:, 11-h:23-h] gives a
            # [128, 12] selector with ones in column h. Columns 12-23 are
            # all-zero; [0:1, 12:24] serves as a zero lhsT for PSUM init.
            eband = constp.tile([128, 24], bf16, tag="eband")
            nc.vector.memset(eband[:], 0.0)
            nc.vector.memset(eband[:, 11:12], 1.0)

            qk_quad = {}
            v_pairs = {}

            def attention(p, par, xt_ref):
                """Scores/attn/normalize/proj for image pair p (quad slot par)."""
                v_sb = v_pairs.pop(p)
                ao_list = []
                sums_ps = sumsp.tile([12, F2], f32, tag="sums",
                                     padded_shape=[12, 512], name=f"sums{p}")
                # zero the sums bank (sets has_written) so the per-unit sums
                # matmuls can accumulate in any interleaving with start=False
                nc.tensor.matmul(
                    sums_ps[:], lhsT=eband[0:1, 12:24],
                    rhs=cos_sb[0:1, 0:F2], start=True, stop=False,
                    skip_group_check=True)
                for hp in range(6):
                    qro = qk_quad[hp + 100]
                    kro = qk_quad[hp + 6 + 100]
                    ao = aops.tile([128, F2], f32, tag="aops",
                                   padded_shape=[128, 512], name=f"ao{p}{hp}")
                    for ic in range(2):
                        qoff = (par * 2 + ic) * N
                        psA = scps.tile([128, F2], f32, tag="psA",
                                        padded_shape=[128, 512], name=f"psA{p}{hp}{ic}")
                        psB = scps.tile([128, F2], f32, tag="psB",
                                        padded_shape=[128, 512], name=f"psB{p}{hp}{ic}")
                        for ph in range(2):
                            h = 2 * hp + ph
                            cr = ph * N
                            qv = qro[ph * 64:(ph + 1) * 64, qoff:qoff + N]
                            nc.tensor.matmul(
                                psA[:, cr:cr + N],
                                lhsT=kro[ph * 64:(ph + 1) * 64, qoff:qoff + 128],
                                rhs=qv, start=True, stop=True)
                            nc.tensor.matmul(
                                psB[0:N1, cr:cr + N],
                                lhsT=kro[ph * 64:(ph + 1) * 64, qoff + 128:qoff + N],
                                rhs=qv, start=True, stop=True)
                        prA = probsp.tile([128, F2], bf16, tag="prA",
                                          name=f"prA{p}{hp}{ic}")
                        prB = probsp.tile([128, F2], bf16, tag="prB",
                                          name=f"prB{p}{hp}{ic}")
                        nc.scalar.activation(prA[:], psA[:], AF.Exp)
                        nc.scalar.activation(prB[0:N1, :], psB[0:N1, :], AF.Exp)
                        # rel-pos bias: probs *= exp(rpb), folded via bf16 DVE
                        nc.vector.tensor_mul(prA[:], prA[:], erpb_sb[hp][0])
                        nc.vector.tensor_mul(prB[0:N1, :], prB[0:N1, :],
                                             erpb_sb[hp][1][0:N1, :])
                        for ph in range(2):
                            h = 2 * hp + ph
                            cr = ph * N
                            nc.tensor.matmul(
                                ao[ph * 64:(ph + 1) * 64, ic * N:(ic + 1) * N],
                                lhsT=v_sb[ic][0][:, h * 64:(h + 1) * 64],
                                rhs=prA[:, cr:cr + N], start=True, stop=False)
                            nc.tensor.matmul(
                                ao[ph * 64:(ph + 1) * 64, ic * N:(ic + 1) * N],
                                lhsT=v_sb[ic][1][0:N1, h * 64:(h + 1) * 64],
                                rhs=prB[0:N1, cr:cr + N], start=False, stop=True)
                            last = (hp == 5 and ic == 1 and ph == 1)
                            nc.tensor.matmul(
                                sums_ps[:, ic * N:(ic + 1) * N],
                                lhsT=eband[:, 11 - h:23 - h],
                                rhs=prA[:, cr:cr + N],
                                start=False, stop=False, skip_group_check=True)
                            nc.tensor.matmul(
                                sums_ps[:, ic * N:(ic + 1) * N],
                                lhsT=eband[0:N1, 11 - h:23 - h],
                                rhs=prB[0:N1, cr:cr + N],
                                start=False, stop=last, skip_group_check=True)
                    aot = aosbp.tile([128, F2], f32, tag="aosb",
                                     name=f"aot{p}{hp}")
                    nc.scalar.activation(aot[:], ao[:], AF.Copy)
                    ao_list.append(aot)

                # ---- normalization: r = exp(-ln(sums)) ----
                lnt = rsbp.tile([12, F2], f32, tag="lnt", name=f"lnt{p}")
                nc.scalar.activation(lnt[:], sums_ps[:], AF.Ln)
                rsm = rsbp.tile([12, F2], f32, tag="rsm", name=f"rsm{p}")
                nc.scalar.activation(rsm[:], lnt[:], AF.Exp, scale=-1.0)
                ot_sb = []
                for hp in range(6):
                    # broadcast r rows across partitions via 0-stride DMA
                    rb = rbc.tile([128, F2], f32, tag="rbc", name=f"rb{p}{hp}")
                    nc.gpsimd.dma_start(
                        rb[0:64, :],
                        rsm[2 * hp:2 * hp + 1, :].unsqueeze(1)
                        .broadcast_to((1, 64, F2)))
                    nc.gpsimd.dma_start(
                        rb[64:128, :],
                        rsm[2 * hp + 1:2 * hp + 2, :].unsqueeze(1)
                        .broadcast_to((1, 64, F2)))
                    ot = otp.tile([128, F2], f32r, tag="ot", name=f"ot{p}{hp}")
                    nc.vector.tensor_mul(ot[:], ao_list[hp][:], rb[:])
                    ot_sb.append(ot)

                # ---- output projection (bias added at eviction) ----
                for ic in range(2):
                    img = (2 * p + ic) % n_img
                    for n_off, n_sz in ((0, N0), (N0, N1)):
                        yt = ysbp.tile([128, C], f32, tag="ysb",
                                       name=f"yt{p}{ic}{n_off}")
                        for ch in range(2):
                            ps = yps.tile([128, 384], f32, tag="yps",
                                          padded_shape=[128, 512],
                                          name=f"yps{p}{ic}{n_off}{ch}")
                            for cc in range(6):
                                nc.tensor.matmul(
                                    ps[0:n_sz, :],
                                    lhsT=ot_sb[cc][:, ic * N + n_off:ic * N + n_off + n_sz],
                                    rhs=projt_sb[cc][:, ch * 384:(ch + 1) * 384],
                                    start=(cc == 0), stop=(cc == 5))
                            nc.vector.tensor_add(
                                yt[0:n_sz, ch * 384:(ch + 1) * 384],
                                ps[0:n_sz, :],
                                projb_bc[0:n_sz, ch * 384:(ch + 1) * 384])
                        nc.sync.dma_start(
                            y_d[img, n_off:n_off + n_sz, :], yt[0:n_sz, :])

            total_pairs = repeat * n_pairs
            for pi in range(total_pairs):
                p = pi % n_pairs
                par = pi % 2
                # ---- load xT for this image pair ----
                xt_sb = []
                for cc in range(6):
                    t = xtp.tile([128, F2], f32r, tag=f"xt{cc}", name=f"xt{pi}{cc}")
                    nc.sync.dma_start(
                        t[:], xt_d[p, cc * 128:(cc + 1) * 128, :].bitcast(f32r))
                    xt_sb.append(t)

                # ---- q/k projection into quad tiles ----
                if par == 0:
                    for m in range(12):
                        qk_quad[m] = qkbfp.tile(
                            [128, F4], bf16, tag=f"qk{m}", name=f"qk{pi}{m}")
                for m in range(12):
                    ps = qkps.tile([128, F2], f32, tag="qkps",
                                   padded_shape=[128, 512], name=f"qkp{pi}{m}")
                    for cc in range(6):
                        nc.tensor.matmul(
                            ps[:],
                            lhsT=wt_sb[cc][:, m * 128:(m + 1) * 128],
                            rhs=xt_sb[cc][:],
                            start=(cc == 0), stop=(cc == 5))
                    dst = qk_quad[m][:, par * F2:(par + 1) * F2]
                    if m < 6:
                        nc.scalar.activation(dst, ps[:], AF.Copy)
                    else:
                        nc.vector.tensor_copy(dst, ps[:])

                # ---- v projection (natural out) ----
                v_sb = []
                for ic in range(2):
                    vts = [vsbp.tile([128, C], bf16, tag="vsb",
                                     name=f"vsb{pi}{ic}{i}") for i in range(2)]
                    for nck, (n_off, n_sz) in enumerate(((0, N0), (N0, N1))):
                        for ch in range(2):
                            ps = vps.tile([128, 384], f32, tag="vps",
                                          padded_shape=[128, 512],
                                          name=f"vps{pi}{ic}{nck}{ch}")
                            for cc in range(6):
                                nc.tensor.matmul(
                                    ps[0:n_sz, :],
                                    lhsT=xt_sb[cc][:, ic * N + n_off:ic * N + n_off + n_sz],
                                    rhs=wt_sb[cc][:, 2 * C + ch * 384:2 * C + (ch + 1) * 384],
                                    start=(cc == 0), stop=(cc == 5))
                            nc.scalar.activation(
                                vts[nck][0:n_sz, ch * 384:(ch + 1) * 384],
                                ps[0:n_sz, :], AF.Copy)
                    v_sb.append(vts)
                v_pairs[p] = v_sb

                # ---- rope on the full quad, then attention for both pairs ----
                if par == 1 or pi == total_pairs - 1:
                    fw = F4 if par == 1 else F2
                    for m in range(12):
                        src = qk_quad[m]
                        qs = ropet.tile([128, F4], bf16, tag="qs", name=f"qs{pi}{m}")
                        nc.vector.stream_shuffle(qs[:, 0:fw], src[:, 0:fw], SHUF_MASK)
                        u = ropet.tile([128, F4], bf16, tag="u", name=f"u{pi}{m}")
                        v = ropet.tile([128, F4], bf16, tag="v", name=f"v{pi}{m}")
                        # (src + bq) * cos on all m (bq columns 6-11 are zero
                        # for the k blocks) — TensorScalarPtr hits 4x DVE mode
                        # with all-bf16 operands.
                        nc.vector.scalar_tensor_tensor(
                            out=u[:, 0:fw], in0=src[:, 0:fw],
                            scalar=bq_sb[:, m:m + 1],
                            in1=cos_sb[:, 0:fw], op0=OP.add, op1=OP.mult)
                        nc.vector.scalar_tensor_tensor(
                            out=v[:, 0:fw], in0=qs[:, 0:fw],
                            scalar=bqs_sb[:, m:m + 1],
                            in1=spm_sb[:, 0:fw], op0=OP.add, op1=OP.mult)
                        nc.vector.tensor_add(src[:, 0:fw], u[:, 0:fw], v[:, 0:fw])
                        qk_quad[m + 100] = src
                    if par == 1:
                        attention((pi - 1) % n_pairs, 0, None)
                    attention(p, par, None)
    nc.compile()
    return nc


def host_prepare(inputs):
    x = np.asarray(inputs["x"], np.float32)
    qkv_w = np.asarray(inputs["qkv_w"], np.float32)
    scale = D ** -0.5
    Wq = qkv_w[:C] + np.asarray(inputs["lora_q_b"]) @ np.asarray(inputs["lora_q_a"])
    Wk = qkv_w[C:2 * C] + np.asarray(inputs["lora_k_b"]) @ np.asarray(inputs["lora_k_a"])
    Wv = qkv_w[2 * C:] + np.asarray(inputs["lora_v_b"]) @ np.asarray(inputs["lora_v_a"])
    p64 = _perm64()
    perm = (np.arange(H)[:, None] * D + p64[None, :]).ravel()
    Wq_de = (Wq * scale)[perm]
    bq_de = (np.asarray(inputs["q_bias"], np.float32) * scale)[perm]
    Wk_de = Wk[perm]
    wt = np.ascontiguousarray(np.concatenate([Wq_de, Wk_de, Wv], 0).T)

    bq = np.zeros((128, 12), np.float32)
    bq[:, 0:6] = bq_de.reshape(6, 128).T
    bqs = np.zeros((128, 12), np.float32)
    bqs[:, 0:6] = np.stack(
        [_swap16_rows(bq_de[i * 128:(i + 1) * 128]) for i in range(6)], 1)

    cos_f = np.ones((N, D), np.float32)
    cos_f[1:] = np.asarray(inputs["rope_cos"], np.float32)
    sin_f = np.zeros((N, D), np.float32)
    sin_f[1:] = np.asarray(inputs["rope_sin"], np.float32)
    cos_de = np.ascontiguousarray(cos_f[:, p64].T)
    spm = np.ascontiguousarray(sin_f[:, p64].T)
    for blk in range(2):
        spm[blk * 32:blk * 32 + 16] *= -1.0
    cs = np.stack([
        np.tile(np.vstack([cos_de, cos_de]), (1, 4)),
        np.tile(np.vstack([spm, spm]), (1, 4)),
    ]).astype(ml_dtypes.bfloat16)

    rel_table = np.asarray(inputs["rel_table"], np.float32)
    rel_index = np.asarray(inputs["rel_index"])
    rpb = rel_table[rel_index.reshape(-1)].reshape(N, N, H)
    rpbT = rpb.transpose(2, 1, 0)  # [h, j, i]
    erpb = np.exp(rpbT)  # probs multiplier: exp(s + rpb) = exp(s) * exp(rpb)
    erpbt = np.ones((6, 2, 128, F2), np.float32)
    for hp in range(6):
        for ph in range(2):
            h = 2 * hp + ph
            erpbt[hp, 0, :, ph * N:(ph + 1) * N] = erpb[h, 0:128, :]
            erpbt[hp, 1, 0:N1, ph * N:(ph + 1) * N] = erpb[h, 128:N, :]
    erpbt = erpbt.astype(ml_dtypes.bfloat16)

    proj_w = np.asarray(inputs["proj_w"], np.float32)
    projt = np.ascontiguousarray(proj_w.T)
    projb = (np.asarray(inputs["proj_b"], np.float32)
             + proj_w @ np.asarray(inputs["v_bias"], np.float32)).reshape(1, C)

    xt = x.transpose(0, 2, 1)  # [B, C, N]
    xt_pairs = np.ascontiguousarray(
        xt.reshape(B // 2, 2, C, N).transpose(0, 2, 1, 3).reshape(B // 2, C, 2 * N))

    shared = dict(wt=wt, bq=bq, bqs=bqs, cs=cs, erpbt=erpbt,
                  projt=projt, projb=projb)
    per_core = []
    ppc = BPC // 2
    for c in range(NCORES):
        m = dict(shared)
        m["xt"] = np.ascontiguousarray(xt_pairs[c * ppc:(c + 1) * ppc])
        per_core.append(m)
    return per_core


def kernel(**inputs):
    from concourse.bass_utils import run_bass_kernel_spmd
    in_maps = host_prepare(inputs)
    if "nc" not in _cache:
        _cache["nc"] = build_program()
    nc = _cache["nc"]
    res = run_bass_kernel_spmd(nc, in_maps, list(range(NCORES))).results
    y = np.concatenate([res[c]["y"] for c in range(NCORES)], 0)
    return np.ascontiguousarray(y.astype(np.float32))



# revision 31
# speedup vs baseline: 1.0326x; 1.0326x over previous
"""EVA-02 ViT attention block (LoRA + rope + rel-pos-bias) on 8 TRN2 NeuronCores.

Data-parallel over batch (8 images per core). Per core:
  - LoRA merged into qkv weights on the host; q-scale and v-bias folded away.
  - q/k projected in transposed layout (channels on partitions), v natural,
    fp32r matmuls at full PE rate (free dim >= 256 via image pairs).
  - q-bias added during PSUM eviction (Act Identity+bias); rope is pure
    bf16 tensor ops on DVE (2x mode), pair-swap via stream_shuffle.
  - scores transposed (S^T[j,i]). Head ph=0 of each pair: single matmul
    (start=True works at PE base partition 0) and rel-pos bias applied as
    probs *= exp(rpb) on DVE. Head ph=1 (lhsT at base partition 64, where
    a start=True matmul faults): identity-matmul rpb opener + accumulate,
    as in the classic scheme.
  - exp on ScalarE without max subtraction (scores are O(1)); probs bf16.
  - attn@v with v stationary emits O^T; softmax denominators via
    ones-vector matmuls; 1/x via DVE reciprocal_approx_fast (avoids the
    Ln/Exp act-table ping-pong); normalization applied in the O^T
    eviction multiply with a batched 2-DMA broadcast of the reciprocals.
  - output projection back to natural [n, c] layout; bias on DVE eviction.
"""
import numpy as np
import ml_dtypes

B, N, C, H, R = 64, 197, 768, 12, 24
D = C // H               # 64
NCORES = 8
BPC = B // NCORES        # images per core
F2 = 2 * N               # 394
F4 = 4 * N               # 788
N0, N1 = 128, N - 128    # token chunks: 128 + 69

_cache = {}

SHUF_MASK = list(range(16, 32)) + list(range(0, 16))
ROPE_ORDER = [0, 6, 1, 7, 2, 8, 3, 9, 4, 10, 5, 11]


def _perm64():
    p = []
    for blk in range(2):
        base = blk * 32
        p += [base + 2 * t for t in range(16)]
        p += [base + 2 * t + 1 for t in range(16)]
    return np.array(p)


def build_program(n_pairs=BPC // 2, use_shuffle=True, repeat=1):
    import concourse.bass as bass
    import concourse.tile as tile
    from concourse import bacc, mybir

    f32, f32r, bf16 = mybir.dt.float32, mybir.dt.float32r, mybir.dt.bfloat16
    AF = mybir.ActivationFunctionType
    OP = mybir.AluOpType

    nc = bacc.Bacc("TRN2", target_bir_lowering=False, debug=False)
    n_img = 2 * n_pairs

    xt_d = nc.dram_tensor("xt", [n_pairs, C, F2], bf16, kind="ExternalInput")
    wt_d = nc.dram_tensor("wt", [C, 3 * C], bf16, kind="ExternalInput")
    bq_d = nc.dram_tensor("bq", [128, 6], f32, kind="ExternalInput")
    cs_d = nc.dram_tensor("cs", [2, 128, F4], bf16, kind="ExternalInput")
    rpbo_d = nc.dram_tensor("rpbo", [6, 2, 128, N], bf16, kind="ExternalInput")
    erpe_d = nc.dram_tensor("erpe", [6, 2, 128, N], bf16, kind="ExternalInput")
    ident_d = nc.dram_tensor("ident", [128, 128], bf16, kind="ExternalInput")
    projt_d = nc.dram_tensor("projt", [C, C], bf16, kind="ExternalInput")
    projb_d = nc.dram_tensor("projb", [1, C], f32, kind="ExternalInput")
    y_d = nc.dram_tensor("y", [n_img, N, C], f32, kind="ExternalOutput")
    # DRAM bounce buffer for the softmax-reciprocal broadcast (SBUF->SBUF
    # DMAs cannot replicate one partition row to many)
    rsf_d = nc.dram_tensor("rsf_scratch", [2, 2, 6 * F2], f32, kind="Internal")

    from contextlib import ExitStack
    with tile.TileContext(nc) as tc:
        with ExitStack() as stk:
            pool = lambda name, bufs, **kw: stk.enter_context(
                tc.tile_pool(name=name, bufs=bufs, **kw))
            # NOTE: bufs is per-tag. PSUM budget: qkps 2 + vps 1 + psA 1 +
            # psB 1 + aops 1 + sums 1 + yps 1 = 8 banks exactly.
            constp = pool("const", 1)
            xtp = pool("xt", 2)
            qkps = pool("qkps", 2, space="PSUM")
            vps = pool("vps", 1, space="PSUM")
            qkbfp = pool("qkbf", 2)
            ropet = pool("ropet", 1)
            vsbp = pool("vsb", 8)
            scps = pool("scps", 1, space="PSUM")
            probsp = pool("probs", 4)
            aops = pool("aops", 1, space="PSUM")
            sumsp = pool("sums", 1, space="PSUM")
            rsbp = pool("rsb", 2)
            aosbp = pool("aosb", 12)
            rbc = pool("rbc", 1)
            yps = pool("yps", 1, space="PSUM")
            ysbp = pool("ysb", 2)
            otp = pool("otp", 6)

            # ---- constants (batched DMAs) ----
            wt_all = constp.tile([128, 6 * 3 * C], bf16, tag="wtall")
            nc.sync.dma_start(
                wt_all[:].rearrange("p (cc j) -> p cc j", cc=6),
                wt_d.rearrange("(cc p) j -> cc p j", cc=6)
                .transpose((1, 0, 2)))
            wt_sb = [wt_all[:, cc * 3 * C:(cc + 1) * 3 * C] for cc in range(6)]
            pt_all = constp.tile([128, 6 * C], bf16, tag="ptall")
            nc.sync.dma_start(
                pt_all[:].rearrange("p (cc j) -> p cc j", cc=6),
                projt_d.rearrange("(cc p) j -> cc p j", cc=6)
                .transpose((1, 0, 2)))
            projt_sb = [pt_all[:, cc * C:(cc + 1) * C] for cc in range(6)]
            rpbo_all = constp.tile([128, 12 * N], bf16, tag="rpboall")
            nc.sync.dma_start(
                rpbo_all[:].rearrange("p (g j) -> p g j", g=12),
                rpbo_d.rearrange("h c p j -> (h c) p j").transpose((1, 0, 2)))
            rpbo_sb = [(rpbo_all[:, (2 * hp) * N:(2 * hp + 1) * N],
                        rpbo_all[:, (2 * hp + 1) * N:(2 * hp + 2) * N])
                       for hp in range(6)]
            erpe_all = constp.tile([128, 12 * N], bf16, tag="erpeall")
            nc.sync.dma_start(
                erpe_all[:].rearrange("p (g j) -> p g j", g=12),
                erpe_d.rearrange("h c p j -> (h c) p j").transpose((1, 0, 2)))
            erpe_sb = [(erpe_all[:, (2 * hp) * N:(2 * hp + 1) * N],
                        erpe_all[:, (2 * hp + 1) * N:(2 * hp + 2) * N])
                       for hp in range(6)]
            projb_bc = constp.tile([128, C], f32, tag="pbbc")
            nc.gpsimd.dma_start(
                projb_bc[:],
                projb_d[:].unsqueeze(1).broadcast_to((1, 128, C)))
            bq_sb = constp.tile([128, 6], f32, tag="bq")
            nc.sync.dma_start(bq_sb[:], bq_d[:])
            cos_sb = constp.tile([128, F4], bf16, tag="cos")
            nc.sync.dma_start(cos_sb[:], cs_d[0])
            spm_sb = constp.tile([128, F4], bf16, tag="spm")
            nc.sync.dma_start(spm_sb[:], cs_d[1])
            ident_sb = constp.tile([128, 128], bf16, tag="ident")
            nc.sync.dma_start(ident_sb[:], ident_d[:])
            # E-band: column 11 is ones; slicing [:, 11-h:23-h] gives a
            # [128, 12] selector with ones in column h. Columns 12-23 are
            # all-zero; [0:1, 12:24] serves as a zero lhsT for PSUM init.
            eband = constp.tile([128, 24], bf16, tag="eband")
            nc.vector.memset(eband[:], 0.0)
            nc.vector.memset(eband[:, 11:12], 1.0)

            qk_quad = {}
            v_pairs = {}
            att_state = {}

            def attention_phase1(p, par, rope_unit=None):
                """Scores/probs/attn@v/sums for image pair p (quad slot par).

                rope_unit(k), when given, emits the rope for m=k, k+6; units
                are interleaved into the head loop so the DVE FIFO serves
                attention's probs multiplies between rope units instead of
                after all of them.
                """
                v_sb = v_pairs.pop(p)
                ao_list = []
                sums_ps = sumsp.tile([12, F2], f32, tag="sums",
                                     padded_shape=[12, 512], name=f"sums{p}")
                # zero the sums bank (sets has_written) so the per-unit sums
                # matmuls can accumulate in any interleaving with start=False
                nc.tensor.matmul(
                    sums_ps[:], lhsT=eband[0:1, 12:24],
                    rhs=cos_sb[0:1, 0:F2], start=True, stop=False,
                    skip_group_check=True)
                if rope_unit is not None:
                    rope_unit(0)
                for hp in range(6):
                    qro = qk_quad[hp + 100]
                    kro = qk_quad[hp + 6 + 100]
                    ao = aops.tile([128, F2], f32, tag="aops",
                                   padded_shape=[128, 512], name=f"ao{p}{hp}")
                    for ic in range(2):
                        qoff = (par * 2 + ic) * N
                        psA = scps.tile([128, F2], f32, tag="psA",
                                        padded_shape=[128, 512], name=f"psA{p}{hp}{ic}")
                        psB = scps.tile([128, F2], f32, tag="psB",
                                        padded_shape=[128, 512], name=f"psB{p}{hp}{ic}")
                        # ph=0: plain matmul, bias folded via exp(rpb) later.
                        # ph=1 (base partition 64): a start=True matmul
                        # faults there, so open with the identity-rpb add.
                        qv0 = qro[0:64, qoff:qoff + N]
                        nc.tensor.matmul(
                            psA[:, 0:N], lhsT=kro[0:64, qoff:qoff + 128],
                            rhs=qv0, start=True, stop=True)
                        nc.tensor.matmul(
                            psB[0:N1, 0:N], lhsT=kro[0:64, qoff + 128:qoff + N],
                            rhs=qv0, start=True, stop=True)
                        qv1 = qro[64:128, qoff:qoff + N]
                        nc.tensor.matmul(
                            psA[:, N:F2], lhsT=ident_sb[:],
                            rhs=rpbo_sb[hp][0], start=True, stop=False)
                        nc.tensor.matmul(
                            psA[:, N:F2], lhsT=kro[64:128, qoff:qoff + 128],
                            rhs=qv1, start=False, stop=True)
                        nc.tensor.matmul(
                            psB[0:N1, N:F2], lhsT=ident_sb[0:N1, 0:N1],
                            rhs=rpbo_sb[hp][1][0:N1, :], start=True, stop=False)
                        nc.tensor.matmul(
                            psB[0:N1, N:F2], lhsT=kro[64:128, qoff + 128:qoff + N],
                            rhs=qv1, start=False, stop=True)
                        prA = probsp.tile([128, F2], bf16, tag="prA",
                                          name=f"prA{p}{hp}{ic}")
                        prB = probsp.tile([128, F2], bf16, tag="prB",
                                          name=f"prB{p}{hp}{ic}")
                        nc.scalar.activation(prA[:], psA[:], AF.Exp)
                        nc.scalar.activation(prB[0:N1, :], psB[0:N1, :], AF.Exp)
                        # keep the DVE fed: next rope unit goes ahead of the
                        # probs multiplies it does not depend on
                        if rope_unit is not None and ic == 0 and hp < 5:
                            rope_unit(hp + 1)
                        # rel-pos bias for head ph=0: probs *= exp(rpb).
                        # On Pool (gpsimd): slower per-op but off the DVE
                        # FIFO, which the rope keeps saturated.
                        nc.gpsimd.tensor_mul(prA[:, 0:N], prA[:, 0:N],
                                             erpe_sb[hp][0])
                        nc.gpsimd.tensor_mul(prB[0:N1, 0:N], prB[0:N1, 0:N],
                                             erpe_sb[hp][1][0:N1, :])
                        for ph in range(2):
                            h = 2 * hp + ph
                            cr = ph * N
                            nc.tensor.matmul(
                                ao[ph * 64:(ph + 1) * 64, ic * N:(ic + 1) * N],
                                lhsT=v_sb[ic][0][:, h * 64:(h + 1) * 64],
                                rhs=prA[:, cr:cr + N], start=True, stop=False)
                            nc.tensor.matmul(
                                ao[ph * 64:(ph + 1) * 64, ic * N:(ic + 1) * N],
                                lhsT=v_sb[ic][1][0:N1, h * 64:(h + 1) * 64],
                                rhs=prB[0:N1, cr:cr + N], start=False, stop=True)
                            last = (hp == 5 and ic == 1 and ph == 1)
                            nc.tensor.matmul(
                                sums_ps[:, ic * N:(ic + 1) * N],
                                lhsT=eband[:, 11 - h:23 - h],
                                rhs=prA[:, cr:cr + N],
                                start=False, stop=False, skip_group_check=True)
                            nc.tensor.matmul(
                                sums_ps[:, ic * N:(ic + 1) * N],
                                lhsT=eband[0:N1, 11 - h:23 - h],
                                rhs=prB[0:N1, cr:cr + N],
                                start=False, stop=last, skip_group_check=True)
                    aot = aosbp.tile([128, F2], bf16, tag="aosb",
                                     name=f"aot{p}{hp}")
                    nc.scalar.activation(aot[:], ao[:], AF.Copy)
                    ao_list.append(aot)
                att_state[p] = (ao_list, sums_ps)

            def attention_phase2(p):
                """Normalize (1/sums broadcast multiply) + output projection."""
                ao_list, sums_ps = att_state.pop(p)
                # ---- normalization: r = 1/sums via fast DVE reciprocal ----
                rsf = rsbp.tile([12, F2], f32, tag="rsf", name=f"rsf{p}")
                nc.vector.reciprocal_approx_fast(rsf[:], sums_ps[:])
                # broadcast r rows across partitions via a DRAM bounce:
                # heads 2hp -> rows 0-63, heads 2hp+1 -> rows 64-127
                slot = p % 2
                # store half-major: dram[half, hp*F2 + i] = rsf[2*hp + half, i]
                nc.gpsimd.dma_start(
                    rsf_d[slot].rearrange("h (g i) -> g h i", g=6), rsf[:])
                rball = rbc.tile([128, 6 * F2], f32, tag="rbc", name=f"rb{p}")
                for half in range(2):
                    nc.gpsimd.dma_start(
                        rball[half * 64:(half + 1) * 64, :],
                        rsf_d[slot, half:half + 1].unsqueeze(1)
                        .broadcast_to((1, 64, 6 * F2)))
                ot_sb = []
                for hp in range(6):
                    ot = otp.tile([128, F2], bf16, tag="ot", name=f"ot{p}{hp}")
                    nc.vector.tensor_mul(ot[:], ao_list[hp][:],
                                         rball[:, hp * F2:(hp + 1) * F2])
                    ot_sb.append(ot)

                # ---- output projection (bias added at eviction) ----
                for ic in range(2):
                    img = (2 * p + ic) % n_img
                    for n_off, n_sz in ((0, N0), (N0, N1)):
                        yt = ysbp.tile([128, C], f32, tag="ysb",
                                       name=f"yt{p}{ic}{n_off}")
                        for ch in range(2):
                            ps = yps.tile([128, 384], f32, tag="yps",
                                          padded_shape=[128, 512],
                                          name=f"yps{p}{ic}{n_off}{ch}")
                            for cc in range(6):
                                nc.tensor.matmul(
                                    ps[0:n_sz, :],
                                    lhsT=ot_sb[cc][:, ic * N + n_off:ic * N + n_off + n_sz],
                                    rhs=projt_sb[cc][:, ch * 384:(ch + 1) * 384],
                                    start=(cc == 0), stop=(cc == 5))
                            nc.vector.tensor_add(
                                yt[0:n_sz, ch * 384:(ch + 1) * 384],
                                ps[0:n_sz, :],
                                projb_bc[0:n_sz, ch * 384:(ch + 1) * 384])
                        nc.sync.dma_start(
                            y_d[img, n_off:n_off + n_sz, :], yt[0:n_sz, :])

            total_pairs = repeat * n_pairs
            for pi in range(total_pairs):
                p = pi % n_pairs
                par = pi % 2
                # ---- load xT for this image pair ----
                xt_sb = []
                for cc in range(6):
                    t = xtp.tile([128, F2], bf16, tag=f"xt{cc}", name=f"xt{pi}{cc}")
                    nc.sync.dma_start(
                        t[:], xt_d[p, cc * 128:(cc + 1) * 128, :])
                    xt_sb.append(t)

                # ---- q/k projection into quad tiles ----
                if par == 0:
                    qkall = qkbfp.tile([128, 12 * F4], bf16, tag="qkall",
                                       name=f"qkall{pi}")
                    qk_quad["all"] = qkall
                    for m in range(12):
                        qk_quad[m] = qkall[:, m * F4:(m + 1) * F4]
                for m in range(12):
                    ps = qkps.tile([128, F2], f32, tag="qkps",
                                   padded_shape=[128, 512], name=f"qkp{pi}{m}")
                    for cc in range(6):
                        nc.tensor.matmul(
                            ps[:],
                            lhsT=wt_sb[cc][:, m * 128:(m + 1) * 128],
                            rhs=xt_sb[cc][:],
                            start=(cc == 0), stop=(cc == 5))
                    dst = qk_quad[m][:, par * F2:(par + 1) * F2]
                    if m < 6:
                        # q eviction adds the (scaled, permuted) q bias so
                        # rope needs no scalar term
                        nc.scalar.activation(dst, ps[:], AF.Identity,
                                             bias=bq_sb[:, m:m + 1])
                    else:
                        nc.vector.tensor_copy(dst, ps[:])

                # ---- v projection (natural out) ----
                v_sb = []
                for ic in range(2):
                    vts = [vsbp.tile([128, C], bf16, tag="vsb",
                                     name=f"vsb{pi}{ic}{i}") for i in range(2)]
                    for nck, (n_off, n_sz) in enumerate(((0, N0), (N0, N1))):
                        for ch in range(2):
                            ps = vps.tile([128, 384], f32, tag="vps",
                                          padded_shape=[128, 512],
                                          name=f"vps{pi}{ic}{nck}{ch}")
                            for cc in range(6):
                                nc.tensor.matmul(
                                    ps[0:n_sz, :],
                                    lhsT=xt_sb[cc][:, ic * N + n_off:ic * N + n_off + n_sz],
                                    rhs=wt_sb[cc][:, 2 * C + ch * 384:2 * C + (ch + 1) * 384],
                                    start=(cc == 0), stop=(cc == 5))
                            nc.scalar.activation(
                                vts[nck][0:n_sz, ch * 384:(ch + 1) * 384],
                                ps[0:n_sz, :], AF.Copy)
                    v_sb.append(vts)
                v_pairs[p] = v_sb

                # ---- rope + attention, software-pipelined over the quad ----
                if par == 1 or pi == total_pairs - 1:
                    fw = F4 if par == 1 else F2
                    # pair-swap via SBUF->SBUF DMA (swaps 16-row halves of
                    # each 32-partition block), in 3-m column chunks so rope
                    # math can start before the whole quad is copied
                    qkall = qk_quad["all"]
                    qsf = ropet.tile([128, 12 * F4], bf16, tag="qsf",
                                     name=f"qsf{pi}")
                    qk_v = qkall[:].rearrange("(b h s) c -> b h s c", h=2, s=16)
                    qs_v = qsf[:].rearrange("(b h s) c -> b h s c", h=2, s=16)
                    for mc in range(0, 12, 3):
                        c0, c1 = mc * F4, (mc + 3) * F4
                        for h in range(2):
                            nc.gpsimd.dma_start(
                                qs_v[:, h, :, c0:c1],
                                qk_v[:, 1 - h, :, c0:c1])

                    def rope_unit(k, pi=pi, fw=fw):
                        """Rope m=k (q) and m=k+6 (k) of the current quad."""
                        for m in (k, k + 6):
                            src = qk_quad[m]
                            qs = qsf[:, m * F4:(m + 1) * F4]
                            u = ropet.tile([128, F4], bf16, tag="u",
                                           name=f"u{pi}{m}")
                            v = ropet.tile([128, F4], bf16, tag="v",
                                           name=f"v{pi}{m}")
                            # bias already folded at eviction; all-bf16
                            # TensorTensor ops run in the 2x DVE mode
                            nc.vector.tensor_mul(u[:, 0:fw], src[:, 0:fw],
                                                 cos_sb[:, 0:fw])
                            nc.vector.tensor_mul(v[:, 0:fw], qs[0:128, 0:fw],
                                                 spm_sb[:, 0:fw])
                            nc.vector.tensor_add(src[:, 0:fw], u[:, 0:fw],
                                                 v[:, 0:fw])
                            qk_quad[m + 100] = src

                    prev = (pi - 1) % n_pairs
                    if par == 1:
                        attention_phase1(prev, 0, rope_unit)
                        attention_phase1(p, par)
                        attention_phase2(prev)
                        attention_phase2(p)
                    else:
                        attention_phase1(p, par, rope_unit)
                        attention_phase2(p)
    nc.compile()
    return nc


def host_prepare(inputs):
    x = np.asarray(inputs["x"], np.float32)
    qkv_w = np.asarray(inputs["qkv_w"], np.float32)
    scale = D ** -0.5
    Wq = qkv_w[:C] + np.asarray(inputs["lora_q_b"]) @ np.asarray(inputs["lora_q_a"])
    Wk = qkv_w[C:2 * C] + np.asarray(inputs["lora_k_b"]) @ np.asarray(inputs["lora_k_a"])
    Wv = qkv_w[2 * C:] + np.asarray(inputs["lora_v_b"]) @ np.asarray(inputs["lora_v_a"])
    p64 = _perm64()
    perm = (np.arange(H)[:, None] * D + p64[None, :]).ravel()
    Wq_de = (Wq * scale)[perm]
    bq_de = (np.asarray(inputs["q_bias"], np.float32) * scale)[perm]
    Wk_de = Wk[perm]
    wt = np.ascontiguousarray(
        np.concatenate([Wq_de, Wk_de, Wv], 0).T).astype(ml_dtypes.bfloat16)

    bq = np.ascontiguousarray(bq_de.reshape(6, 128).T)

    cos_f = np.ones((N, D), np.float32)
    cos_f[1:] = np.asarray(inputs["rope_cos"], np.float32)
    sin_f = np.zeros((N, D), np.float32)
    sin_f[1:] = np.asarray(inputs["rope_sin"], np.float32)
    cos_de = np.ascontiguousarray(cos_f[:, p64].T)
    spm = np.ascontiguousarray(sin_f[:, p64].T)
    for blk in range(2):
        spm[blk * 32:blk * 32 + 16] *= -1.0
    cs = np.stack([
        np.tile(np.vstack([cos_de, cos_de]), (1, 4)),
        np.tile(np.vstack([spm, spm]), (1, 4)),
    ]).astype(ml_dtypes.bfloat16)

    rel_table = np.asarray(inputs["rel_table"], np.float32)
    rel_index = np.asarray(inputs["rel_index"])
    rpb = rel_table[rel_index.reshape(-1)].reshape(N, N, H)
    rpbT = rpb.transpose(2, 1, 0)  # [h, j, i]
    # odd heads keep the additive bias (identity-matmul opener); even heads
    # get exp(rpb) as a probs multiplier
    rpbo = np.zeros((6, 2, 128, N), ml_dtypes.bfloat16)
    erpe = np.ones((6, 2, 128, N), np.float32)
    for hp in range(6):
        rpbo[hp, 0] = rpbT[2 * hp + 1, 0:128, :].astype(ml_dtypes.bfloat16)
        rpbo[hp, 1, 0:N1] = rpbT[2 * hp + 1, 128:N, :].astype(ml_dtypes.bfloat16)
        erpe[hp, 0] = np.exp(rpbT[2 * hp, 0:128, :])
        erpe[hp, 1, 0:N1] = np.exp(rpbT[2 * hp, 128:N, :])
    erpe = erpe.astype(ml_dtypes.bfloat16)

    ident = np.eye(128, dtype=ml_dtypes.bfloat16)
    proj_w = np.asarray(inputs["proj_w"], np.float32)
    projt = np.ascontiguousarray(proj_w.T).astype(ml_dtypes.bfloat16)
    projb = (np.asarray(inputs["proj_b"], np.float32)
             + proj_w @ np.asarray(inputs["v_bias"], np.float32)).reshape(1, C)

    xt = x.transpose(0, 2, 1)  # [B, C, N]
    xt_pairs = np.ascontiguousarray(
        xt.reshape(B // 2, 2, C, N).transpose(0, 2, 1, 3)
        .reshape(B // 2, C, 2 * N)).astype(ml_dtypes.bfloat16)

    shared = dict(wt=wt, bq=bq, cs=cs, rpbo=rpbo, erpe=erpe, ident=ident,
                  projt=projt, projb=projb)
    per_core = []
    ppc = BPC // 2
    for c in range(NCORES):
        m = dict(shared)
        m["xt"] = np.ascontiguousarray(xt_pairs[c * ppc:(c + 1) * ppc])
        per_core.append(m)
    return per_core


def kernel(**inputs):
    from concourse.bass_utils import run_bass_kernel_spmd
    in_maps = host_prepare(inputs)
    if "nc" not in _cache:
        _cache["nc"] = build_program()
    nc = _cache["nc"]
    res = run_bass_kernel_spmd(nc, in_maps, list(range(NCORES))).results
    y = np.concatenate([res[c]["y"] for c in range(NCORES)], 0)
    return np.ascontiguousarray(y.astype(np.float32))


# revision 35
# speedup vs baseline: 1.4893x; 1.4422x over previous
"""EVA-02 ViT attention block (LoRA + rope + rel-pos-bias) on 8 TRN2 NeuronCores.

Data-parallel over batch (8 images per core). Per core:
  - LoRA merged into qkv weights on the host; q-scale and v-bias folded away.
  - q/k projected in transposed layout (channels on partitions), v natural,
    fp32r matmuls at full PE rate (free dim >= 256 via image pairs).
  - q-bias added during PSUM eviction (Act Identity+bias); rope is pure
    bf16 tensor ops on DVE (2x mode), pair-swap via stream_shuffle.
  - scores transposed (S^T[j,i]). Head ph=0 of each pair: single matmul
    (start=True works at PE base partition 0) and rel-pos bias applied as
    probs *= exp(rpb) on DVE. Head ph=1 (lhsT at base partition 64, where
    a start=True matmul faults): identity-matmul rpb opener + accumulate,
    as in the classic scheme.
  - exp on ScalarE without max subtraction (scores are O(1)); probs bf16.
  - attn@v with v stationary emits O^T; softmax denominators via
    ones-vector matmuls; 1/x via DVE reciprocal_approx_fast (avoids the
    Ln/Exp act-table ping-pong); normalization applied in the O^T
    eviction multiply with a batched 2-DMA broadcast of the reciprocals.
  - output projection back to natural [n, c] layout; bias on DVE eviction.
"""
import numpy as np
import ml_dtypes

B, N, C, H, R = 64, 197, 768, 12, 24
D = C // H               # 64
NCORES = 8
BPC = B // NCORES        # images per core
F2 = 2 * N               # 394
F4 = 4 * N               # 788
N0, N1 = 128, N - 128    # token chunks: 128 + 69

_cache = {}

SHUF_MASK = list(range(16, 32)) + list(range(0, 16))
ROPE_ORDER = [0, 6, 1, 7, 2, 8, 3, 9, 4, 10, 5, 11]


def _perm64():
    p = []
    for blk in range(2):
        base = blk * 32
        p += [base + 2 * t for t in range(16)]
        p += [base + 2 * t + 1 for t in range(16)]
    return np.array(p)


def build_program(n_pairs=BPC // 2, use_shuffle=True, repeat=1):
    import concourse.bass as bass
    import concourse.tile as tile
    from concourse import bacc, mybir

    f32, f32r, bf16 = mybir.dt.float32, mybir.dt.float32r, mybir.dt.bfloat16
    AF = mybir.ActivationFunctionType
    OP = mybir.AluOpType

    nc = bacc.Bacc("TRN2", target_bir_lowering=False, debug=False)
    n_img = 2 * n_pairs

    xt_d = nc.dram_tensor("xt", [n_pairs, C, F2], bf16, kind="ExternalInput")
    wt_d = nc.dram_tensor("wt", [C, 3 * C], bf16, kind="ExternalInput")
    bq_d = nc.dram_tensor("bq", [128, 6], f32, kind="ExternalInput")
    cs_d = nc.dram_tensor("cs", [2, 128, F4], bf16, kind="ExternalInput")
    erpe_d = nc.dram_tensor("erpe", [6, 2, 128, F2], bf16, kind="ExternalInput")
    projt_d = nc.dram_tensor("projt", [C, C], bf16, kind="ExternalInput")
    projb_d = nc.dram_tensor("projb", [1, C], f32, kind="ExternalInput")
    y_d = nc.dram_tensor("y", [n_img, N, C], f32, kind="ExternalOutput")
    # DRAM bounce buffer for the softmax-reciprocal broadcast (SBUF->SBUF
    # DMAs cannot replicate one partition row to many)
    rsf_d = nc.dram_tensor("rsf_scratch", [2, 2, 6 * F2], f32, kind="Internal")

    from contextlib import ExitStack
    with tile.TileContext(nc) as tc:
        with ExitStack() as stk:
            pool = lambda name, bufs, **kw: stk.enter_context(
                tc.tile_pool(name=name, bufs=bufs, **kw))
            # NOTE: bufs is per-tag. PSUM budget: qkps 2 + vps 1 + psA 1 +
            # psB 1 + aops 1 + sums 1 + yps 1 = 8 banks exactly.
            constp = pool("const", 1)
            xtp = pool("xt", 2)
            qkps = pool("qkps", 2, space="PSUM")
            qkbfp = pool("qkbf", 2)
            ropet = pool("ropet", 1)
            vsbp = pool("vsb", 8)
            scps = pool("scps", 1, space="PSUM")
            probsp = pool("probs", 4)
            aops = pool("aops", 1, space="PSUM")
            sumsp = pool("sums", 1, space="PSUM")
            rsbp = pool("rsb", 2)
            aosbp = pool("aosb", 12)
            rbc = pool("rbc", 1)
            yps = pool("yps", 2, space="PSUM")
            ysbp = pool("ysb", 2)
            otp = pool("otp", 6)

            # ---- constants (batched DMAs) ----
            wt_all = constp.tile([128, 6 * 3 * C], bf16, tag="wtall")
            wt_v = wt_all[:].rearrange("p (cc j) -> p cc j", cc=6)
            wt_dv = wt_d.rearrange("(cc p) j -> cc p j", cc=6).transpose((1, 0, 2))
            nc.sync.dma_start(wt_v[:, :, 0:2 * C], wt_dv[:, :, 0:2 * C])
            nc.sync.dma_start(wt_v[:, :, 2 * C:3 * C], wt_dv[:, :, 2 * C:3 * C])
            wt_sb = [wt_all[:, cc * 3 * C:(cc + 1) * 3 * C] for cc in range(6)]
            pt_all = constp.tile([128, 6 * C], bf16, tag="ptall")
            nc.sync.dma_start(
                pt_all[:].rearrange("p (cc j) -> p cc j", cc=6),
                projt_d.rearrange("(cc p) j -> cc p j", cc=6)
                .transpose((1, 0, 2)))
            projt_sb = [pt_all[:, cc * C:(cc + 1) * C] for cc in range(6)]
            erpe_all = constp.tile([128, 12 * F2], bf16, tag="erpeall")
            nc.sync.dma_start(
                erpe_all[:].rearrange("p (g j) -> p g j", g=12),
                erpe_d.rearrange("h c p j -> (h c) p j").transpose((1, 0, 2)))
            erpe_sb = [(erpe_all[:, (2 * hp) * F2:(2 * hp + 1) * F2],
                        erpe_all[:, (2 * hp + 1) * F2:(2 * hp + 2) * F2])
                       for hp in range(6)]
            projb_bc = constp.tile([128, C], f32, tag="pbbc")
            nc.gpsimd.dma_start(
                projb_bc[:],
                projb_d[:].unsqueeze(1).broadcast_to((1, 128, C)))
            bq_sb = constp.tile([128, 6], f32, tag="bq")
            nc.sync.dma_start(bq_sb[:], bq_d[:])
            cos_sb = constp.tile([128, F4], bf16, tag="cos")
            nc.sync.dma_start(cos_sb[:], cs_d[0])
            spm_sb = constp.tile([128, F4], bf16, tag="spm")
            nc.sync.dma_start(spm_sb[:], cs_d[1])
            # E-band: column 11 is ones; slicing [:, 11-h:23-h] gives a
            # [128, 12] selector with ones in column h. Columns 12-23 are
            # all-zero; [0:1, 12:24] serves as a zero lhsT for PSUM init.
            eband = constp.tile([128, 24], bf16, tag="eband")
            nc.vector.memset(eband[:], 0.0)
            nc.vector.memset(eband[:, 11:12], 1.0)

            qk_quad = {}
            v_pairs = {}
            att_state = {}

            def attention_phase1(p, par, rope_unit=None):
                """Scores/probs/attn@v/sums for image pair p (quad slot par).

                rope_unit(k), when given, emits the rope for m=k, k+6; units
                are interleaved into the head loop so the DVE FIFO serves
                attention's probs multiplies between rope units instead of
                after all of them.
                """
                v_sb = v_pairs.pop(p)
                ao_list = []
                sums_ps = sumsp.tile([12, F2], f32, tag="sums",
                                     padded_shape=[12, 512], name=f"sums{p}")
                # zero the sums bank (sets has_written) so the per-unit sums
                # matmuls can accumulate in any interleaving with start=False
                nc.tensor.matmul(
                    sums_ps[:], lhsT=eband[0:1, 12:24],
                    rhs=cos_sb[0:1, 0:F2], start=True, stop=False,
                    skip_group_check=True)
                if rope_unit is not None:
                    rope_unit(0)
                for hp in range(6):
                    qro = qk_quad[hp + 100]
                    kro = qk_quad[hp + 6 + 100]
                    ao = aops.tile([128, F2], f32, tag="aops",
                                   padded_shape=[128, 512], name=f"ao{p}{hp}")
                    for ic in range(2):
                        qoff = (par * 2 + ic) * N
                        # per-bank PE-tile rule: all start=True openers in
                        # one PSUM bank must share the lhsT base partition,
                        # so ph=0 groups live in ps0 and ph=1 groups in ps1
                        # (cols 0 and 256, both 1KB-aligned)
                        ps0 = scps.tile([128, 512], f32, tag="ps0",
                                        padded_shape=[128, 512], name=f"ps0{p}{hp}{ic}")
                        ps1 = scps.tile([128, 512], f32, tag="ps1",
                                        padded_shape=[128, 512], name=f"ps1{p}{hp}{ic}")
                        qv0 = qro[0:64, qoff:qoff + N]
                        nc.tensor.matmul(
                            ps0[:, 0:N], lhsT=kro[0:64, qoff:qoff + 128],
                            rhs=qv0, start=True, stop=True)
                        nc.tensor.matmul(
                            ps0[0:N1, 256:256 + N],
                            lhsT=kro[0:64, qoff + 128:qoff + N],
                            rhs=qv0, start=True, stop=True)
                        qv1 = qro[64:128, qoff:qoff + N]
                        nc.tensor.matmul(
                            ps1[:, 0:N], lhsT=kro[64:128, qoff:qoff + 128],
                            rhs=qv1, start=True, stop=True)
                        nc.tensor.matmul(
                            ps1[0:N1, 256:256 + N],
                            lhsT=kro[64:128, qoff + 128:qoff + N],
                            rhs=qv1, start=True, stop=True)
                        pr = probsp.tile([128, 2 * F2], bf16, tag="pr",
                                         name=f"pr{p}{hp}{ic}")
                        prA = pr[:, 0:F2]
                        prB = pr[0:128, F2:2 * F2]
                        # one exp per score bank: strided AP covers the A
                        # chunk and the B chunk (B rows >= N1 read stale
                        # psum, land in unread probs rows)
                        for ph, bank in ((0, ps0), (1, ps1)):
                            nc.scalar.activation(
                                pr[:].rearrange("q (c x i) -> q c x i",
                                                c=2, x=2)[:, :, ph, :],
                                bank[:].rearrange("q (c z) -> q c z",
                                                  c=2)[:, :, 0:N],
                                AF.Exp)
                        # keep the DVE fed: next rope unit goes ahead of the
                        # probs multiplies it does not depend on
                        if rope_unit is not None and ic == 0 and hp < 5:
                            rope_unit(hp + 1)
                        # rel-pos bias: probs *= exp(rpb); prA on Pool (off
                        # the rope-saturated DVE FIFO), small prB on DVE
                        nc.gpsimd.tensor_mul(prA, prA, erpe_sb[hp][0])
                        nc.vector.tensor_mul(prB[0:N1, :], prB[0:N1, :],
                                             erpe_sb[hp][1][0:N1, :])
                        for ph in range(2):
                            h = 2 * hp + ph
                            cr = ph * N
                            nc.tensor.matmul(
                                ao[ph * 64:(ph + 1) * 64, ic * N:(ic + 1) * N],
                                lhsT=v_sb[ic][0][:, h * 64:(h + 1) * 64],
                                rhs=prA[:, cr:cr + N], start=True, stop=False)
                            nc.tensor.matmul(
                                ao[ph * 64:(ph + 1) * 64, ic * N:(ic + 1) * N],
                                lhsT=v_sb[ic][1][0:N1, h * 64:(h + 1) * 64],
                                rhs=prB[0:N1, cr:cr + N], start=False, stop=True)
                            last = (hp == 5 and ic == 1 and ph == 1)
                            nc.tensor.matmul(
                                sums_ps[:, ic * N:(ic + 1) * N],
                                lhsT=eband[:, 11 - h:23 - h],
                                rhs=prA[:, cr:cr + N],
                                start=False, stop=False, skip_group_check=True)
                            nc.tensor.matmul(
                                sums_ps[:, ic * N:(ic + 1) * N],
                                lhsT=eband[0:N1, 11 - h:23 - h],
                                rhs=prB[0:N1, cr:cr + N],
                                start=False, stop=last, skip_group_check=True)
                    aot = aosbp.tile([128, F2], bf16, tag="aosb",
                                     name=f"aot{p}{hp}")
                    nc.scalar.activation(aot[:], ao[:], AF.Copy)
                    ao_list.append(aot)
                att_state[p] = (ao_list, sums_ps)

            def attention_phase2(p):
                """Normalize (1/sums broadcast multiply) + output projection."""
                ao_list, sums_ps = att_state.pop(p)
                # ---- normalization: r = 1/sums via fast DVE reciprocal ----
                rsf = rsbp.tile([12, F2], f32, tag="rsf", name=f"rsf{p}")
                nc.vector.reciprocal_approx_fast(rsf[:], sums_ps[:])
                # broadcast r rows across partitions via a DRAM bounce:
                # heads 2hp -> rows 0-63, heads 2hp+1 -> rows 64-127
                slot = p % 2
                # store half-major: dram[half, hp*F2 + i] = rsf[2*hp + half, i]
                nc.gpsimd.dma_start(
                    rsf_d[slot].rearrange("h (g i) -> g h i", g=6), rsf[:])
                rball = rbc.tile([128, 6 * F2], f32, tag="rbc", name=f"rb{p}")
                for half in range(2):
                    nc.gpsimd.dma_start(
                        rball[half * 64:(half + 1) * 64, :],
                        rsf_d[slot, half:half + 1].unsqueeze(1)
                        .broadcast_to((1, 64, 6 * F2)))
                ot_sb = []
                for hp in range(6):
                    ot = otp.tile([128, F2], bf16, tag="ot", name=f"ot{p}{hp}")
                    nc.vector.tensor_mul(ot[:], ao_list[hp][:],
                                         rball[:, hp * F2:(hp + 1) * F2])
                    ot_sb.append(ot)

                # ---- output projection (bias added at eviction) ----
                for ic in range(2):
                    img = (2 * p + ic) % n_img
                    for n_off, n_sz in ((0, N0), (N0, N1)):
                        yt = ysbp.tile([128, C], f32, tag="ysb",
                                       name=f"yt{p}{ic}{n_off}")
                        for ch in range(2):
                            ps = yps.tile([128, 384], f32, tag="yps",
                                          padded_shape=[128, 512],
                                          name=f"yps{p}{ic}{n_off}{ch}")
                            for cc in range(6):
                                nc.tensor.matmul(
                                    ps[0:n_sz, :],
                                    lhsT=ot_sb[cc][:, ic * N + n_off:ic * N + n_off + n_sz],
                                    rhs=projt_sb[cc][:, ch * 384:(ch + 1) * 384],
                                    start=(cc == 0), stop=(cc == 5))
                            nc.vector.tensor_add(
                                yt[0:n_sz, ch * 384:(ch + 1) * 384],
                                ps[0:n_sz, :],
                                projb_bc[0:n_sz, ch * 384:(ch + 1) * 384])
                        nc.scalar.dma_start(
                            y_d[img, n_off:n_off + n_sz, :], yt[0:n_sz, :])

            total_pairs = repeat * n_pairs
            xt_pre = {}

            def load_xt(pi):
                if pi in xt_pre or pi >= total_pairs:
                    return
                pp = pi % n_pairs
                tiles = []
                for cc in range(6):
                    t = xtp.tile([128, F2], bf16, tag=f"xt{cc}", name=f"xt{pi}{cc}")
                    nc.sync.dma_start(
                        t[:], xt_d[pp, cc * 128:(cc + 1) * 128, :])
                    tiles.append(t)
                xt_pre[pi] = tiles

            for pi in range(total_pairs):
                p = pi % n_pairs
                par = pi % 2
                load_xt(pi)
                xt_sb = xt_pre.pop(pi)

                # ---- q/k projection into quad tiles ----
                if par == 0:
                    for m in range(12):
                        qk_quad[m] = qkbfp.tile(
                            [128, F4], bf16, tag=f"qk{m}", name=f"qk{pi}{m}")
                for m in range(12):
                    ps = qkps.tile([128, F2], f32, tag="qkps",
                                   padded_shape=[128, 512], name=f"qkp{pi}{m}")
                    for cc in range(6):
                        nc.tensor.matmul(
                            ps[:],
                            lhsT=wt_sb[cc][:, m * 128:(m + 1) * 128],
                            rhs=xt_sb[cc][:],
                            start=(cc == 0), stop=(cc == 5))
                    dst = qk_quad[m][:, par * F2:(par + 1) * F2]
                    if m < 6:
                        # q eviction adds the (scaled, permuted) q bias so
                        # rope needs no scalar term
                        nc.scalar.activation(dst, ps[:], AF.Identity,
                                             bias=bq_sb[:, m:m + 1])
                    else:
                        nc.vector.tensor_copy(dst, ps[:])

                # ---- v projection (natural out) ----
                v_sb = []
                for ic in range(2):
                    vts = [vsbp.tile([128, C], bf16, tag="vsb",
                                     name=f"vsb{pi}{ic}{i}") for i in range(2)]
                    for nck, (n_off, n_sz) in enumerate(((0, N0), (N0, N1))):
                        for ch in range(2):
                            ps = yps.tile([128, 384], f32, tag="yps",
                                          padded_shape=[128, 512],
                                          name=f"vps{pi}{ic}{nck}{ch}")
                            for cc in range(6):
                                nc.tensor.matmul(
                                    ps[0:n_sz, :],
                                    lhsT=xt_sb[cc][:, ic * N + n_off:ic * N + n_off + n_sz],
                                    rhs=wt_sb[cc][:, 2 * C + ch * 384:2 * C + (ch + 1) * 384],
                                    start=(cc == 0), stop=(cc == 5))
                            nc.scalar.activation(
                                vts[nck][0:n_sz, ch * 384:(ch + 1) * 384],
                                ps[0:n_sz, :], AF.Copy)
                    v_sb.append(vts)
                v_pairs[p] = v_sb
                load_xt(pi + 1)  # prefetch next pair while attention runs

                # ---- rope + attention, software-pipelined over the quad ----
                if par == 1 or pi == total_pairs - 1:
                    fw = F4 if par == 1 else F2

                    def rope_unit(k, pi=pi, fw=fw):
                        """Rope m=k (q) and m=k+6 (k) of the current quad."""
                        for m in (k, k + 6):
                            src = qk_quad[m]
                            qs = ropet.tile([128, F4], bf16, tag="qs",
                                            name=f"qs{pi}{m}")
                            nc.vector.stream_shuffle(qs[:, 0:fw], src[:, 0:fw],
                                                     SHUF_MASK)
                            u = ropet.tile([128, F4], bf16, tag="u",
                                           name=f"u{pi}{m}")
                            v = ropet.tile([128, F4], bf16, tag="v",
                                           name=f"v{pi}{m}")
                            # bias already folded at eviction; all-bf16
                            # TensorTensor ops run in the 2x DVE mode
                            nc.vector.tensor_mul(u[:, 0:fw], src[:, 0:fw],
                                                 cos_sb[:, 0:fw])
                            nc.vector.tensor_mul(v[:, 0:fw], qs[:, 0:fw],
                                                 spm_sb[:, 0:fw])
                            nc.vector.tensor_add(src[:, 0:fw], u[:, 0:fw],
                                                 v[:, 0:fw])
                            qk_quad[m + 100] = src

                    prev = (pi - 1) % n_pairs
                    if par == 1:
                        attention_phase1(prev, 0, rope_unit)
                        attention_phase1(p, par)
                        attention_phase2(prev)
                        attention_phase2(p)
                    else:
                        attention_phase1(p, par, rope_unit)
                        attention_phase2(p)
    nc.compile()
    return nc


def host_prepare(inputs):
    x = np.asarray(inputs["x"], np.float32)
    qkv_w = np.asarray(inputs["qkv_w"], np.float32)
    scale = D ** -0.5
    Wq = qkv_w[:C] + np.asarray(inputs["lora_q_b"]) @ np.asarray(inputs["lora_q_a"])
    Wk = qkv_w[C:2 * C] + np.asarray(inputs["lora_k_b"]) @ np.asarray(inputs["lora_k_a"])
    Wv = qkv_w[2 * C:] + np.asarray(inputs["lora_v_b"]) @ np.asarray(inputs["lora_v_a"])
    p64 = _perm64()
    perm = (np.arange(H)[:, None] * D + p64[None, :]).ravel()
    Wq_de = (Wq * scale)[perm]
    bq_de = (np.asarray(inputs["q_bias"], np.float32) * scale)[perm]
    Wk_de = Wk[perm]
    wt = np.ascontiguousarray(
        np.concatenate([Wq_de, Wk_de, Wv], 0).T).astype(ml_dtypes.bfloat16)

    bq = np.ascontiguousarray(bq_de.reshape(6, 128).T)

    cos_f = np.ones((N, D), np.float32)
    cos_f[1:] = np.asarray(inputs["rope_cos"], np.float32)
    sin_f = np.zeros((N, D), np.float32)
    sin_f[1:] = np.asarray(inputs["rope_sin"], np.float32)
    cos_de = np.ascontiguousarray(cos_f[:, p64].T)
    spm = np.ascontiguousarray(sin_f[:, p64].T)
    for blk in range(2):
        spm[blk * 32:blk * 32 + 16] *= -1.0
    cs = np.stack([
        np.tile(np.vstack([cos_de, cos_de]), (1, 4)),
        np.tile(np.vstack([spm, spm]), (1, 4)),
    ]).astype(ml_dtypes.bfloat16)

    rel_table = np.asarray(inputs["rel_table"], np.float32)
    rel_index = np.asarray(inputs["rel_index"])
    rpb = rel_table[rel_index.reshape(-1)].reshape(N, N, H)
    rpbT = rpb.transpose(2, 1, 0)  # [h, j, i]
    # rel-pos bias as a probs multiplier exp(rpb) for all heads
    erpe = np.ones((6, 2, 128, F2), np.float32)
    for hp in range(6):
        for ph in range(2):
            h = 2 * hp + ph
            erpe[hp, 0, :, ph * N:(ph + 1) * N] = np.exp(rpbT[h, 0:128, :])
            erpe[hp, 1, 0:N1, ph * N:(ph + 1) * N] = np.exp(rpbT[h, 128:N, :])
    erpe = erpe.astype(ml_dtypes.bfloat16)

    proj_w = np.asarray(inputs["proj_w"], np.float32)
    projt = np.ascontiguousarray(proj_w.T).astype(ml_dtypes.bfloat16)
    projb = (np.asarray(inputs["proj_b"], np.float32)
             + proj_w @ np.asarray(inputs["v_bias"], np.float32)).reshape(1, C)

    xt = x.transpose(0, 2, 1)  # [B, C, N]
    xt_pairs = np.ascontiguousarray(
        xt.reshape(B // 2, 2, C, N).transpose(0, 2, 1, 3)
        .reshape(B // 2, C, 2 * N)).astype(ml_dtypes.bfloat16)

    shared = dict(wt=wt, bq=bq, cs=cs, erpe=erpe,
                  projt=projt, projb=projb)
    per_core = []
    ppc = BPC // 2
    for c in range(NCORES):
        m = dict(shared)
        m["xt"] = np.ascontiguousarray(xt_pairs[c * ppc:(c + 1) * ppc])
        per_core.append(m)
    return per_core


def kernel(**inputs):
    from concourse.bass_utils import run_bass_kernel_spmd
    in_maps = host_prepare(inputs)
    if "nc" not in _cache:
        _cache["nc"] = build_program()
    nc = _cache["nc"]
    res = run_bass_kernel_spmd(nc, in_maps, list(range(NCORES))).results
    y = np.concatenate([res[c]["y"] for c in range(NCORES)], 0)
    return np.ascontiguousarray(y.astype(np.float32))


# revision 37
# speedup vs baseline: 2.1185x; 1.4225x over previous
"""EVA-02 ViT attention block (LoRA + rope + rel-pos-bias) on 8 TRN2 NeuronCores.

Data-parallel over batch (8 images per core), all matmuls bf16 (1 cyc/row).
Per core:
  - LoRA merged into qkv weights on the host; q-scale and v-bias folded away.
  - q/k projected in transposed layout (channels on partitions), v natural.
  - q-bias added during PSUM eviction (Act Identity+bias); rope is pure
    bf16 tensor ops on DVE (2x mode), pair-swap via stream_shuffle.
  - scores transposed (S^T[j,i]), one matmul per (head, j-chunk). Walrus
    quadrant-tile constraint: all start=True openers within one PSUM bank
    must share the lhsT base partition, so ph=0 heads (base 0) and ph=1
    heads (base 64) accumulate in separate banks, at 1KB-aligned offsets.
  - rel-pos bias folded multiplicatively: probs *= exp(rpb) (host table),
    A-chunk on Pool, B-chunk on DVE — off the rope-saturated DVE FIFO.
  - exp on ScalarE without max subtraction (scores are O(1)); probs bf16,
    both j-chunks of a head-pair written by one strided-AP activation.
  - attn@v with v stationary emits O^T; softmax denominators via
    ones-vector matmuls; 1/x via DVE reciprocal_approx_fast (avoids the
    Ln/Exp act-table ping-pong); reciprocals broadcast to all partitions
    through a DRAM bounce (SBUF->SBUF DMA cannot replicate rows).
  - per-quad software pipeline: rope units interleaved into the first
    attention's head loop; phase1 (scores..sums) of both pairs runs before
    phase2 (normalize + output projection) so PE never waits on the
    normalization chain; xt prefetched one pair ahead; first-use-ordered
    constant DMAs shorten the cold start.
"""
import numpy as np
import ml_dtypes

B, N, C, H, R = 64, 197, 768, 12, 24
D = C // H               # 64
NCORES = 8
BPC = B // NCORES        # images per core
F2 = 2 * N               # 394
F4 = 4 * N               # 788
N0, N1 = 128, N - 128    # token chunks: 128 + 69

_cache = {}

SHUF_MASK = list(range(16, 32)) + list(range(0, 16))
ROPE_ORDER = [0, 6, 1, 7, 2, 8, 3, 9, 4, 10, 5, 11]


def _perm64():
    p = []
    for blk in range(2):
        base = blk * 32
        p += [base + 2 * t for t in range(16)]
        p += [base + 2 * t + 1 for t in range(16)]
    return np.array(p)


def build_program(n_pairs=BPC // 2, use_shuffle=True, repeat=1):
    import concourse.bass as bass
    import concourse.tile as tile
    from concourse import bacc, mybir

    f32, f32r, bf16 = mybir.dt.float32, mybir.dt.float32r, mybir.dt.bfloat16
    AF = mybir.ActivationFunctionType
    OP = mybir.AluOpType

    nc = bacc.Bacc("TRN2", target_bir_lowering=False, debug=False)
    n_img = 2 * n_pairs

    xt_d = nc.dram_tensor("xt", [n_pairs, C, F2], bf16, kind="ExternalInput")
    wt_d = nc.dram_tensor("wt", [C, 3 * C], bf16, kind="ExternalInput")
    bq_d = nc.dram_tensor("bq", [128, 6], f32, kind="ExternalInput")
    cs_d = nc.dram_tensor("cs", [2, 128, F4], bf16, kind="ExternalInput")
    erpe_d = nc.dram_tensor("erpe", [6, 2, 128, F2], bf16, kind="ExternalInput")
    projt_d = nc.dram_tensor("projt", [C, C], bf16, kind="ExternalInput")
    projb_d = nc.dram_tensor("projb", [1, C], f32, kind="ExternalInput")
    y_d = nc.dram_tensor("y", [n_img, N, C], f32, kind="ExternalOutput")
    # DRAM bounce buffer for the softmax-reciprocal broadcast (SBUF->SBUF
    # DMAs cannot replicate one partition row to many)
    rsf_d = nc.dram_tensor("rsf_scratch", [2, 2, 6 * F2], f32, kind="Internal")

    from contextlib import ExitStack
    with tile.TileContext(nc) as tc:
        with ExitStack() as stk:
            pool = lambda name, bufs, **kw: stk.enter_context(
                tc.tile_pool(name=name, bufs=bufs, **kw))
            # NOTE: bufs is per-tag. PSUM budget: qkps 2 + vps 1 + psA 1 +
            # psB 1 + aops 1 + sums 1 + yps 1 = 8 banks exactly.
            constp = pool("const", 1)
            xtp = pool("xt", 2)
            qkps = pool("qkps", 2, space="PSUM")
            qkbfp = pool("qkbf", 2)
            ropet = pool("ropet", 1)
            vsbp = pool("vsb", 8)
            scps = pool("scps", 1, space="PSUM")
            probsp = pool("probs", 4)
            aops = pool("aops", 1, space="PSUM")
            sumsp = pool("sums", 1, space="PSUM")
            rsbp = pool("rsb", 2)
            aosbp = pool("aosb", 12)
            rbc = pool("rbc", 1)
            yps = pool("yps", 2, space="PSUM")
            ysbp = pool("ysb", 2)
            otp = pool("otp", 6)

            total_pairs = repeat * n_pairs
            xt_pre = {}

            def load_xt(pi):
                if pi in xt_pre or pi >= total_pairs:
                    return
                pp = pi % n_pairs
                tiles = []
                for cc in range(6):
                    t = xtp.tile([128, F2], bf16, tag=f"xt{cc}", name=f"xt{pi}{cc}")
                    nc.sync.dma_start(
                        t[:], xt_d[pp, cc * 128:(cc + 1) * 128, :])
                    tiles.append(t)
                xt_pre[pi] = tiles

            # ---- constants (batched DMAs), ordered by first use: q/k
            # weights and the first x tile gate the very first matmul ----
            wt_all = constp.tile([128, 6 * 3 * C], bf16, tag="wtall")
            wt_v = wt_all[:].rearrange("p (cc j) -> p cc j", cc=6)
            wt_dv = wt_d.rearrange("(cc p) j -> cc p j", cc=6).transpose((1, 0, 2))
            nc.sync.dma_start(wt_v[:, :, 0:2 * C], wt_dv[:, :, 0:2 * C])
            load_xt(0)
            nc.sync.dma_start(wt_v[:, :, 2 * C:3 * C], wt_dv[:, :, 2 * C:3 * C])
            wt_sb = [wt_all[:, cc * 3 * C:(cc + 1) * 3 * C] for cc in range(6)]
            pt_all = constp.tile([128, 6 * C], bf16, tag="ptall")
            nc.sync.dma_start(
                pt_all[:].rearrange("p (cc j) -> p cc j", cc=6),
                projt_d.rearrange("(cc p) j -> cc p j", cc=6)
                .transpose((1, 0, 2)))
            projt_sb = [pt_all[:, cc * C:(cc + 1) * C] for cc in range(6)]
            erpe_all = constp.tile([128, 12 * F2], bf16, tag="erpeall")
            nc.sync.dma_start(
                erpe_all[:].rearrange("p (g j) -> p g j", g=12),
                erpe_d.rearrange("h c p j -> (h c) p j").transpose((1, 0, 2)))
            erpe_sb = [(erpe_all[:, (2 * hp) * F2:(2 * hp + 1) * F2],
                        erpe_all[:, (2 * hp + 1) * F2:(2 * hp + 2) * F2])
                       for hp in range(6)]
            projb_bc = constp.tile([128, C], f32, tag="pbbc")
            nc.gpsimd.dma_start(
                projb_bc[:],
                projb_d[:].unsqueeze(1).broadcast_to((1, 128, C)))
            bq_sb = constp.tile([128, 6], f32, tag="bq")
            nc.sync.dma_start(bq_sb[:], bq_d[:])
            cos_sb = constp.tile([128, F4], bf16, tag="cos")
            nc.sync.dma_start(cos_sb[:], cs_d[0])
            spm_sb = constp.tile([128, F4], bf16, tag="spm")
            nc.sync.dma_start(spm_sb[:], cs_d[1])
            # E-band: column 11 is ones; slicing [:, 11-h:23-h] gives a
            # [128, 12] selector with ones in column h. Columns 12-23 are
            # all-zero; [0:1, 12:24] serves as a zero lhsT for PSUM init.
            eband = constp.tile([128, 24], bf16, tag="eband")
            nc.vector.memset(eband[:], 0.0)
            nc.vector.memset(eband[:, 11:12], 1.0)

            qk_quad = {}
            v_pairs = {}
            att_state = {}

            def attention_phase1(p, par, rope_unit=None):
                """Scores/probs/attn@v/sums for image pair p (quad slot par).

                rope_unit(k), when given, emits the rope for m=k, k+6; units
                are interleaved into the head loop so the DVE FIFO serves
                attention's probs multiplies between rope units instead of
                after all of them.
                """
                v_sb = v_pairs.pop(p)
                ao_list = []
                sums_ps = sumsp.tile([12, F2], f32, tag="sums",
                                     padded_shape=[12, 512], name=f"sums{p}")
                # zero the sums bank (sets has_written) so the per-unit sums
                # matmuls can accumulate in any interleaving with start=False
                nc.tensor.matmul(
                    sums_ps[:], lhsT=eband[0:1, 12:24],
                    rhs=cos_sb[0:1, 0:F2], start=True, stop=False,
                    skip_group_check=True)
                if rope_unit is not None:
                    rope_unit(0)
                for hp in range(6):
                    qro = qk_quad[hp + 100]
                    kro = qk_quad[hp + 6 + 100]
                    ao = aops.tile([128, F2], f32, tag="aops",
                                   padded_shape=[128, 512], name=f"ao{p}{hp}")
                    for ic in range(2):
                        qoff = (par * 2 + ic) * N
                        # per-bank PE-tile rule: all start=True openers in
                        # one PSUM bank must share the lhsT base partition,
                        # so ph=0 groups live in ps0 and ph=1 groups in ps1
                        # (cols 0 and 256, both 1KB-aligned)
                        ps0 = scps.tile([128, 512], f32, tag="ps0",
                                        padded_shape=[128, 512], name=f"ps0{p}{hp}{ic}")
                        ps1 = scps.tile([128, 512], f32, tag="ps1",
                                        padded_shape=[128, 512], name=f"ps1{p}{hp}{ic}")
                        qv0 = qro[0:64, qoff:qoff + N]
                        nc.tensor.matmul(
                            ps0[:, 0:N], lhsT=kro[0:64, qoff:qoff + 128],
                            rhs=qv0, start=True, stop=True)
                        nc.tensor.matmul(
                            ps0[0:N1, 256:256 + N],
                            lhsT=kro[0:64, qoff + 128:qoff + N],
                            rhs=qv0, start=True, stop=True)
                        qv1 = qro[64:128, qoff:qoff + N]
                        nc.tensor.matmul(
                            ps1[:, 0:N], lhsT=kro[64:128, qoff:qoff + 128],
                            rhs=qv1, start=True, stop=True)
                        nc.tensor.matmul(
                            ps1[0:N1, 256:256 + N],
                            lhsT=kro[64:128, qoff + 128:qoff + N],
                            rhs=qv1, start=True, stop=True)
                        pr = probsp.tile([128, 2 * F2], bf16, tag="pr",
                                         name=f"pr{p}{hp}{ic}")
                        prA = pr[:, 0:F2]
                        prB = pr[0:128, F2:2 * F2]
                        # one exp per score bank: strided AP covers the A
                        # chunk and the B chunk (B rows >= N1 read stale
                        # psum, land in unread probs rows)
                        for ph, bank in ((0, ps0), (1, ps1)):
                            nc.scalar.activation(
                                pr[:].rearrange("q (c x i) -> q c x i",
                                                c=2, x=2)[:, :, ph, :],
                                bank[:].rearrange("q (c z) -> q c z",
                                                  c=2)[:, :, 0:N],
                                AF.Exp)
                        # keep the DVE fed: next rope unit goes ahead of the
                        # probs multiplies it does not depend on
                        if rope_unit is not None and ic == 0 and hp < 5:
                            rope_unit(hp + 1)
                        # rel-pos bias: probs *= exp(rpb); prA on Pool (off
                        # the rope-saturated DVE FIFO), small prB on DVE
                        nc.gpsimd.tensor_mul(prA, prA, erpe_sb[hp][0])
                        nc.vector.tensor_mul(prB[0:N1, :], prB[0:N1, :],
                                             erpe_sb[hp][1][0:N1, :])
                        for ph in range(2):
                            h = 2 * hp + ph
                            cr = ph * N
                            nc.tensor.matmul(
                                ao[ph * 64:(ph + 1) * 64, ic * N:(ic + 1) * N],
                                lhsT=v_sb[ic][0][:, h * 64:(h + 1) * 64],
                                rhs=prA[:, cr:cr + N], start=True, stop=False)
                            nc.tensor.matmul(
                                ao[ph * 64:(ph + 1) * 64, ic * N:(ic + 1) * N],
                                lhsT=v_sb[ic][1][0:N1, h * 64:(h + 1) * 64],
                                rhs=prB[0:N1, cr:cr + N], start=False, stop=True)
                            last = (hp == 5 and ic == 1 and ph == 1)
                            nc.tensor.matmul(
                                sums_ps[:, ic * N:(ic + 1) * N],
                                lhsT=eband[:, 11 - h:23 - h],
                                rhs=prA[:, cr:cr + N],
                                start=False, stop=False, skip_group_check=True)
                            nc.tensor.matmul(
                                sums_ps[:, ic * N:(ic + 1) * N],
                                lhsT=eband[0:N1, 11 - h:23 - h],
                                rhs=prB[0:N1, cr:cr + N],
                                start=False, stop=last, skip_group_check=True)
                    aot = aosbp.tile([128, F2], bf16, tag="aosb",
                                     name=f"aot{p}{hp}")
                    nc.scalar.activation(aot[:], ao[:], AF.Copy)
                    ao_list.append(aot)
                att_state[p] = (ao_list, sums_ps)

            def attention_phase2(p):
                """Normalize (1/sums broadcast multiply) + output projection."""
                ao_list, sums_ps = att_state.pop(p)
                # ---- normalization: r = 1/sums via fast DVE reciprocal ----
                rsf = rsbp.tile([12, F2], f32, tag="rsf", name=f"rsf{p}")
                nc.vector.reciprocal_approx_fast(rsf[:], sums_ps[:])
                # broadcast r rows across partitions via a DRAM bounce:
                # heads 2hp -> rows 0-63, heads 2hp+1 -> rows 64-127
                slot = p % 2
                # store half-major: dram[half, hp*F2 + i] = rsf[2*hp + half, i]
                nc.gpsimd.dma_start(
                    rsf_d[slot].rearrange("h (g i) -> g h i", g=6), rsf[:])
                rball = rbc.tile([128, 6 * F2], f32, tag="rbc", name=f"rb{p}")
                for half in range(2):
                    nc.gpsimd.dma_start(
                        rball[half * 64:(half + 1) * 64, :],
                        rsf_d[slot, half:half + 1].unsqueeze(1)
                        .broadcast_to((1, 64, 6 * F2)))
                ot_sb = []
                for hp in range(6):
                    ot = otp.tile([128, F2], bf16, tag="ot", name=f"ot{p}{hp}")
                    nc.vector.tensor_mul(ot[:], ao_list[hp][:],
                                         rball[:, hp * F2:(hp + 1) * F2])
                    ot_sb.append(ot)

                # ---- output projection (bias added at eviction) ----
                for ic in range(2):
                    img = (2 * p + ic) % n_img
                    for n_off, n_sz in ((0, N0), (N0, N1)):
                        yt = ysbp.tile([128, C], f32, tag="ysb",
                                       name=f"yt{p}{ic}{n_off}")
                        for ch in range(2):
                            ps = yps.tile([128, 384], f32, tag="yps",
                                          padded_shape=[128, 512],
                                          name=f"yps{p}{ic}{n_off}{ch}")
                            for cc in range(6):
                                nc.tensor.matmul(
                                    ps[0:n_sz, :],
                                    lhsT=ot_sb[cc][:, ic * N + n_off:ic * N + n_off + n_sz],
                                    rhs=projt_sb[cc][:, ch * 384:(ch + 1) * 384],
                                    start=(cc == 0), stop=(cc == 5))
                            nc.vector.tensor_add(
                                yt[0:n_sz, ch * 384:(ch + 1) * 384],
                                ps[0:n_sz, :],
                                projb_bc[0:n_sz, ch * 384:(ch + 1) * 384])
                        nc.scalar.dma_start(
                            y_d[img, n_off:n_off + n_sz, :], yt[0:n_sz, :])

            for pi in range(total_pairs):
                p = pi % n_pairs
                par = pi % 2
                load_xt(pi)
                xt_sb = xt_pre.pop(pi)

                # ---- q/k projection into quad tiles ----
                if par == 0:
                    for m in range(12):
                        qk_quad[m] = qkbfp.tile(
                            [128, F4], bf16, tag=f"qk{m}", name=f"qk{pi}{m}")
                for m in range(12):
                    ps = qkps.tile([128, F2], f32, tag="qkps",
                                   padded_shape=[128, 512], name=f"qkp{pi}{m}")
                    for cc in range(6):
                        nc.tensor.matmul(
                            ps[:],
                            lhsT=wt_sb[cc][:, m * 128:(m + 1) * 128],
                            rhs=xt_sb[cc][:],
                            start=(cc == 0), stop=(cc == 5))
                    dst = qk_quad[m][:, par * F2:(par + 1) * F2]
                    if m < 6:
                        # q eviction adds the (scaled, permuted) q bias so
                        # rope needs no scalar term
                        nc.scalar.activation(dst, ps[:], AF.Identity,
                                             bias=bq_sb[:, m:m + 1])
                    else:
                        nc.vector.tensor_copy(dst, ps[:])

                # ---- v projection (natural out) ----
                v_sb = []
                for ic in range(2):
                    vts = [vsbp.tile([128, C], bf16, tag="vsb",
                                     name=f"vsb{pi}{ic}{i}") for i in range(2)]
                    for nck, (n_off, n_sz) in enumerate(((0, N0), (N0, N1))):
                        for ch in range(2):
                            ps = yps.tile([128, 384], f32, tag="yps",
                                          padded_shape=[128, 512],
                                          name=f"vps{pi}{ic}{nck}{ch}")
                            for cc in range(6):
                                nc.tensor.matmul(
                                    ps[0:n_sz, :],
                                    lhsT=xt_sb[cc][:, ic * N + n_off:ic * N + n_off + n_sz],
                                    rhs=wt_sb[cc][:, 2 * C + ch * 384:2 * C + (ch + 1) * 384],
                                    start=(cc == 0), stop=(cc == 5))
                            nc.scalar.activation(
                                vts[nck][0:n_sz, ch * 384:(ch + 1) * 384],
                                ps[0:n_sz, :], AF.Copy)
                    v_sb.append(vts)
                v_pairs[p] = v_sb
                load_xt(pi + 1)  # prefetch next pair while attention runs

                # ---- rope + attention, software-pipelined over the quad ----
                if par == 1 or pi == total_pairs - 1:
                    fw = F4 if par == 1 else F2

                    def rope_unit(k, pi=pi, fw=fw):
                        """Rope m=k (q) and m=k+6 (k) of the current quad."""
                        for m in (k, k + 6):
                            src = qk_quad[m]
                            qs = ropet.tile([128, F4], bf16, tag="qs",
                                            name=f"qs{pi}{m}")
                            nc.vector.stream_shuffle(qs[:, 0:fw], src[:, 0:fw],
                                                     SHUF_MASK)
                            u = ropet.tile([128, F4], bf16, tag="u",
                                           name=f"u{pi}{m}")
                            v = ropet.tile([128, F4], bf16, tag="v",
                                           name=f"v{pi}{m}")
                            # bias already folded at eviction; all-bf16
                            # TensorTensor ops run in the 2x DVE mode
                            nc.vector.tensor_mul(u[:, 0:fw], src[:, 0:fw],
                                                 cos_sb[:, 0:fw])
                            nc.vector.tensor_mul(v[:, 0:fw], qs[:, 0:fw],
                                                 spm_sb[:, 0:fw])
                            nc.vector.tensor_add(src[:, 0:fw], u[:, 0:fw],
                                                 v[:, 0:fw])
                            qk_quad[m + 100] = src

                    prev = (pi - 1) % n_pairs
                    if par == 1:
                        attention_phase1(prev, 0, rope_unit)
                        attention_phase1(p, par)
                        attention_phase2(prev)
                        attention_phase2(p)
                    else:
                        attention_phase1(p, par, rope_unit)
                        attention_phase2(p)
    nc.compile()
    return nc


def host_prepare(inputs):
    x = np.asarray(inputs["x"], np.float32)
    qkv_w = np.asarray(inputs["qkv_w"], np.float32)
    scale = D ** -0.5
    Wq = qkv_w[:C] + np.asarray(inputs["lora_q_b"]) @ np.asarray(inputs["lora_q_a"])
    Wk = qkv_w[C:2 * C] + np.asarray(inputs["lora_k_b"]) @ np.asarray(inputs["lora_k_a"])
    Wv = qkv_w[2 * C:] + np.asarray(inputs["lora_v_b"]) @ np.asarray(inputs["lora_v_a"])
    p64 = _perm64()
    perm = (np.arange(H)[:, None] * D + p64[None, :]).ravel()
    Wq_de = (Wq * scale)[perm]
    bq_de = (np.asarray(inputs["q_bias"], np.float32) * scale)[perm]
    Wk_de = Wk[perm]
    wt = np.ascontiguousarray(
        np.concatenate([Wq_de, Wk_de, Wv], 0).T).astype(ml_dtypes.bfloat16)

    bq = np.ascontiguousarray(bq_de.reshape(6, 128).T)

    cos_f = np.ones((N, D), np.float32)
    cos_f[1:] = np.asarray(inputs["rope_cos"], np.float32)
    sin_f = np.zeros((N, D), np.float32)
    sin_f[1:] = np.asarray(inputs["rope_sin"], np.float32)
    cos_de = np.ascontiguousarray(cos_f[:, p64].T)
    spm = np.ascontiguousarray(sin_f[:, p64].T)
    for blk in range(2):
        spm[blk * 32:blk * 32 + 16] *= -1.0
    cs = np.stack([
        np.tile(np.vstack([cos_de, cos_de]), (1, 4)),
        np.tile(np.vstack([spm, spm]), (1, 4)),
    ]).astype(ml_dtypes.bfloat16)

    rel_table = np.asarray(inputs["rel_table"], np.float32)
    rel_index = np.asarray(inputs["rel_index"])
    rpb = rel_table[rel_index.reshape(-1)].reshape(N, N, H)
    rpbT = rpb.transpose(2, 1, 0)  # [h, j, i]
    # rel-pos bias as a probs multiplier exp(rpb) for all heads
    erpe = np.ones((6, 2, 128, F2), np.float32)
    for hp in range(6):
        for ph in range(2):
            h = 2 * hp + ph
            erpe[hp, 0, :, ph * N:(ph + 1) * N] = np.exp(rpbT[h, 0:128, :])
            erpe[hp, 1, 0:N1, ph * N:(ph + 1) * N] = np.exp(rpbT[h, 128:N, :])
    erpe = erpe.astype(ml_dtypes.bfloat16)

    proj_w = np.asarray(inputs["proj_w"], np.float32)
    projt = np.ascontiguousarray(proj_w.T).astype(ml_dtypes.bfloat16)
    projb = (np.asarray(inputs["proj_b"], np.float32)
             + proj_w @ np.asarray(inputs["v_bias"], np.float32)).reshape(1, C)

    xt = x.transpose(0, 2, 1)  # [B, C, N]
    xt_pairs = np.ascontiguousarray(
        xt.reshape(B // 2, 2, C, N).transpose(0, 2, 1, 3)
        .reshape(B // 2, C, 2 * N)).astype(ml_dtypes.bfloat16)

    shared = dict(wt=wt, bq=bq, cs=cs, erpe=erpe,
                  projt=projt, projb=projb)
    per_core = []
    ppc = BPC // 2
    for c in range(NCORES):
        m = dict(shared)
        m["xt"] = np.ascontiguousarray(xt_pairs[c * ppc:(c + 1) * ppc])
        per_core.append(m)
    return per_core


def kernel(**inputs):
    from concourse.bass_utils import run_bass_kernel_spmd
    in_maps = host_prepare(inputs)
    if "nc" not in _cache:
        _cache["nc"] = build_program()
    nc = _cache["nc"]
    res = run_bass_kernel_spmd(nc, in_maps, list(range(NCORES))).results
    y = np.concatenate([res[c]["y"] for c in range(NCORES)], 0)
    return np.ascontiguousarray(y.astype(np.float32))
